# revision 52
# baseline (speedup 1.0000x reference)
"""Trainium2 Bass kernel for nn_MultiHeadAttention (B=4, S=2048, D=1024, H=16).

Sharding: core c in 0..7 handles batch c//2 and heads 8*(c%2) .. +8
(data-parallel over batch x tensor-parallel over heads, Megatron-style:
w_q/w_k/w_v row-split, w_o column-split; per-core partial outputs are
summed pairwise on the host, which also adds b_o).

v2: bf16 operand paths everywhere (DMA/DVE halved, no f32r small-N matmul
penalty), row-tiled score matmuls (two heads concurrently on PE row
halves), exact-causal ctx matmuls (start=True clears the whole PSUM bank,
so the partial diagonal matmul is legal), merged two-head exp via 3D APs,
denominators via reciprocal_approx_fast straight off PSUM plus a DMA
partition-broadcast.
"""

import numpy as np
import concourse.bass as bass
import concourse.tile as tile
from concourse import mybir, bacc

F32 = mybir.dt.float32
F32R = mybir.dt.float32r
BF = mybir.dt.bfloat16
I16 = mybir.dt.int16

# Schraudolph exp on DVE: bf16 bits of exp(s/8) ~= s * 23.0831 + 16250.5
# (128*log2(e)/8 and 128*(127 - 0.043), written as int16, bitcast to bf16).
EXP_A = 23.0831204
EXP_B = 16250.5
import os as _os
CTX_LAG = int(_os.environ.get("MHA_CTX_LAG", "8"))  # deferred-work queue depth
AF = mybir.ActivationFunctionType
ALU = mybir.AluOpType

B, S, D, H = 4, 2048, 1024, 16
DK = D // H          # 64
HC = 8               # heads per core
HD = HC * DK         # 512 head dims per core
KT = S // 128        # 16 k(seq) tiles
NWIN = S // 512      # 4 seq windows of 512
NMT = HD // 128      # 4 M-tiles for QKV projections (= head-pair blocks)
DMT = D // 128       # 8 M-tiles for output projection
VW = HD + HC         # 520: vaug row stride per kt (65 per head)


def build_nc(reps=1, timing=False, phases=(1, 1), probe=()):
    nc = bacc.Bacc(None, target_bir_lowering=False)

    if timing:
        # timing-only build: big IO becomes internal DRAM (no host transfer),
        # tiny dummy external in/out keep the NEFF well-formed
        def declare(name, shape, dtype, isOutput=False):
            return nc.dram_tensor(name, shape, dtype)
        dummy_in = nc.declare_dram_parameter("dummy_in", [128, 128], F32,
                                             isOutput=False)
        dummy_out = nc.declare_dram_parameter("dummy_out", [128, 128], F32,
                                              isOutput=True)
    else:
        declare = nc.declare_dram_parameter

    xt_q = declare("xt_q", [D, S], BF, isOutput=False)
    xt_k = declare("xt_k", [D, S], BF, isOutput=False)
    xt_v = declare("xt_v", [D, S], BF, isOutput=False)
    wqt = declare("wqt", [D, HD], BF, isOutput=False)
    wkt = declare("wkt", [D, HD], BF, isOutput=False)
    wvt = declare("wvt", [D, HD], BF, isOutput=False)
    bqp = declare("bqp", [128, NMT], F32, isOutput=False)
    bkp = declare("bkp", [128, NMT], F32, isOutput=False)
    bvp = declare("bvp", [1, HD], BF, isOutput=False)
    wot = declare("wot", [128, NMT * D], BF, isOutput=False)
    pmat = declare("pmat", [128, 128], BF, isOutput=False)
    cost = declare("cost", [128, S], BF, isOutput=False)
    sint = declare("sint", [128, S], BF, isOutput=False)
    trim = declare("trim", [128, 128], BF, isOutput=False)
    out_pt = declare("out_pt", [D, S], BF, isOutput=True)

    with tile.TileContext(nc) as tc:
      if timing:
          with tc.tile_pool(name="dummy", bufs=1) as dp:
              dt_ = dp.tile([128, 128], F32, tag="dt_")
              nc.sync.dma_start(dt_[:], dummy_in[:])
              nc.sync.dma_start(dummy_out[:], dt_[:])
      for _rep in range(reps):
        with tc.tile_pool(name="pers", bufs=1) as pers:
            qrt = pers.tile([128, NMT * S], BF, tag="qrt")
            krt = pers.tile([128, NMT * S], BF, tag="krt")
            vaug = pers.tile([128, KT * VW], BF, tag="vaug")    # 16*520
            ctxt = pers.tile([128, NMT * S], BF, tag="ctxt")
            tri = pers.tile([128, 128], BF, tag="tri")
            nc.sync.dma_start(tri[:], trim[:])
            pm_sb = pers.tile([128, 128], BF, tag="pm_sb")
            nc.sync.dma_start(pm_sb[:], pmat[:])
            ones_bf = pers.tile([128, 128], BF, tag="ones_bf")
            nc.vector.memset(ones_bf[:], 1.0)
            ones_f = pers.tile([1, 64], F32, tag="ones_f")
            nc.vector.memset(ones_f[:], 1.0)
            ones_r = pers.tile([1, 64], F32R, tag="ones_r")
            nc.vector.tensor_copy(ones_r[:], ones_f[:])
            wo_sb = pers.tile([128, NMT * D], BF, tag="wo_sb")
            nc.scalar.dma_start(wo_sb[:], wot[:])
            cos_sb = pers.tile([128, S], BF, tag="cos_sb")
            nc.sync.dma_start(cos_sb[:], cost[:])
            sin_sb = pers.tile([128, S], BF, tag="sin_sb")
            nc.scalar.dma_start(sin_sb[:], sint[:])

            # ---------------- phase A: projections + rope ----------------
            if phases[0]:
              with tc.tile_pool(name="wts", bufs=1) as wts, \
                   tc.tile_pool(name="xv", bufs=2) as xvp, \
                   tc.tile_pool(name="xq", bufs=2) as xqp, \
                   tc.tile_pool(name="xk", bufs=2) as xkp, \
                   tc.tile_pool(name="tmpA", bufs=4) as tmpA, \
                   tc.tile_pool(name="psA", bufs=2, space="PSUM") as psA, \
                   tc.tile_pool(name="psQ", bufs=2, space="PSUM") as psQ, \
                   tc.tile_pool(name="psP", bufs=2, space="PSUM") as psP:
                wv_sb = wts.tile([128, 8 * HD], BF, tag="wv_sb")
                wq_sb = wts.tile([128, 8 * HD], BF, tag="wq_sb")
                wk_sb = wts.tile([128, 8 * HD], BF, tag="wk_sb")
                for w_sb, wt in ((wv_sb, wvt), (wq_sb, wqt), (wk_sb, wkt)):
                    nc.scalar.dma_start(
                        w_sb[:].rearrange("p (k c) -> p k c", k=8),
                        wt[:].rearrange("(k p) c -> p k c", p=128))
                bv_sb = wts.tile([1, HD], BF, tag="bv_sb")
                nc.sync.dma_start(bv_sb[:], bvp[:])
                bq_sb = wts.tile([128, NMT], F32, tag="bq_sb")
                nc.sync.dma_start(bq_sb[:], bqp[:])
                bk_sb = wts.tile([128, NMT], F32, tag="bk_sb")
                nc.sync.dma_start(bk_sb[:], bkp[:])

                for win in range(NWIN):
                    # ---- V: natural layout [seq, hd] + ones cols ----
                    xv = xvp.tile([128, 8 * 512], BF, tag="xv")
                    dma_v = nc.sync if "one_q" in probe else (nc.scalar if win % 2 else nc.sync)
                    dma_v.dma_start(
                        xv[:].rearrange("p (k c) -> p k c", k=8),
                        xt_v[:, win * 512:(win + 1) * 512].rearrange(
                            "(k p) c -> p k c", p=128))
                    for st in range(4):
                        ps = psA.tile([128, 512], F32, tag="psV")
                        for k in range(8):
                            nc.tensor.matmul(
                                ps[:],
                                xv[:, k * 512 + st * 128: k * 512 + st * 128 + 128],
                                wv_sb[:, k * HD:(k + 1) * HD],
                                start=(k == 0), stop=False)
                        # + b_v broadcast along seq via ones outer product
                        nc.tensor.matmul(ps[:], ones_bf[0:1, 0:128], bv_sb[:],
                                         start=False, stop=True)
                        base = (win * 4 + st) * VW
                        nc.vector.tensor_copy(
                            vaug[:, base: base + VW].rearrange(
                                "p (h c) -> p h c", c=65)[:, :, 0:64],
                            ps[:].rearrange("p (h c) -> p h c", c=64))
                        nc.vector.tensor_copy(
                            vaug[:, base + 64: base + VW: 65],
                            ones_bf[:, 0:HC])

                    # ---- Q and K: transposed layout + rope ----
                    for qk, (xt, xp, w_sb, b_sb, dst) in enumerate((
                            (xt_q, xqp, wq_sb, bq_sb, qrt),
                            (xt_k, xkp, wk_sb, bk_sb, krt))):
                        xq = xp.tile([128, 8 * 512], BF, tag="xq")
                        dma_x = nc.sync if "one_q" in probe else (nc.sync if (win + qk) % 2 else nc.scalar)
                        dma_x.dma_start(
                            xq[:].rearrange("p (k c) -> p k c", k=8),
                            xt[:, win * 512:(win + 1) * 512].rearrange(
                                "(k p) c -> p k c", p=128))
                        for mt in range(NMT):
                            psq = psQ.tile([128, 512], F32, tag="psq")
                            for k in range(8):
                                nc.tensor.matmul(
                                    psq[:],
                                    w_sb[:, k * HD + 128 * mt: k * HD + 128 * mt + 128],
                                    xq[:, k * 512:(k + 1) * 512],
                                    start=(k == 0), stop=(k == 7))
                            qt_sb = tmpA.tile([128, 512], BF, tag="qt_sb")
                            nc.scalar.activation(qt_sb[:], psq[:], AF.Identity,
                                                 bias=b_sb[:, mt:mt + 1],
                                                 scale=1.0)
                            psp = psP.tile([128, 512], F32, tag="psp")
                            nc.tensor.matmul(psp[:], pm_sb[:], qt_sb[:],
                                             start=True, stop=True)
                            t1 = tmpA.tile([128, 512], BF, tag="t1")
                            nc.vector.tensor_tensor(
                                t1[:], qt_sb[:],
                                cos_sb[:, win * 512:(win + 1) * 512], ALU.mult)
                            t2 = tmpA.tile([128, 512], BF, tag="t2")
                            nc.vector.tensor_tensor(
                                t2[:], psp[:],
                                sin_sb[:, win * 512:(win + 1) * 512], ALU.mult)
                            nc.vector.tensor_tensor(
                                dst[:, mt * S + win * 512: mt * S + (win + 1) * 512],
                                t1[:], t2[:], ALU.add)

            # ---------------- phase B: attention + output projection ----------------
            if phases[1]:
              _pb = int(_os.environ.get("MHA_POOL_BUMP", "0"))
              with tc.tile_pool(name="exps", bufs=CTX_LAG + 6) as exps, \
                   tc.tile_pool(name="recp", bufs=2 + _pb) as recp, \
                   tc.tile_pool(name="sgp", bufs=2 + _pb) as sgp, \
                   tc.tile_pool(name="outs", bufs=3 + _pb) as outs, \
                   tc.tile_pool(name="psS", bufs=2, space="PSUM") as psS, \
                   tc.tile_pool(name="psC", bufs=1, space="PSUM") as psC, \
                   tc.tile_pool(name="psB", bufs=2, space="PSUM") as psB:
                # Deferred-work queue: ctx matmuls, den/normalize chains and
                # out-projections are emitted CTX_LAG score-slots late; with
                # "xp" the queue carries across pairs/chunks so PE fills its
                # Act-gated score-stream gaps with the previous pair's work.
                xp = "noxp" not in probe
                pend = []

                def make_ctx(kt, r0, ex, st8, g, ktmax):
                    def emit():
                        if "psc" not in st8:
                            psc_t = psC.tile([65, 1024], F32, tag="psc")
                            st8["psc"] = psc_t
                        psc = st8["psc"]
                        nc.tensor.matmul(
                            psc[0:65, r0:512],
                            vaug[:, kt * VW + 65 * 2 * g:
                                 kt * VW + 65 * 2 * g + 65],
                            ex[:, r0:512],
                            start=(kt == ktmax), stop=(kt == 0))
                        nc.tensor.matmul(
                            psc[0:65, 512 + r0:1024],
                            vaug[:, kt * VW + 65 * (2 * g + 1):
                                 kt * VW + 65 * (2 * g + 1) + 65],
                            ex[:, 512 + r0:1024],
                            start=(kt == ktmax), stop=(kt == 0))
                    return emit

                def make_den(st8, gS, qlo):
                    def emit():
                        psc = st8["psc"]
                        den_r = recp.tile([1, 1024], F32R, tag="den_r")
                        nc.vector.tensor_copy(den_r[:], psc[64:65, :])
                        bcA = psB.tile([64, 512], F32, tag="bc")
                        nc.tensor.matmul(bcA[:], ones_r[:],
                                         den_r[0:1, 0:512],
                                         start=True, stop=True)
                        bcB = psB.tile([64, 512], F32, tag="bc")
                        nc.tensor.matmul(bcB[:], ones_r[:],
                                         den_r[0:1, 512:1024],
                                         start=True, stop=True)
                        rbA = sgp.tile([64, 512], F32, tag="rbA")
                        nc.vector.reciprocal_approx_fast(rbA[:], bcA[:])
                        rbB = sgp.tile([64, 512], F32, tag="rbB")
                        nc.vector.reciprocal_approx_fast(rbB[:], bcB[:])
                        nc.vector.tensor_tensor(
                            ctxt[0:64, gS + qlo: gS + qlo + 512],
                            psc[0:64, 0:512], rbA[:], ALU.mult)
                        sg = sgp.tile([64, 512], BF, tag="sg")
                        nc.vector.tensor_tensor(
                            sg[:], psc[0:64, 512:1024], rbB[:], ALU.mult)
                        nc.sync.dma_start(
                            ctxt[64:128, gS + qlo: gS + qlo + 512], sg[:])
                    return emit

                def make_outproj(qlo):
                    def emit():
                        for mt in range(DMT):
                            pso_full = psS.tile([128, 1024], F32, tag="pss")
                            pso = pso_full[:, 0:512]
                            for g in range(NMT):
                                nc.tensor.matmul(
                                    pso[:],
                                    wo_sb[:, g * D + 128 * mt:
                                          g * D + 128 * mt + 128],
                                    ctxt[:, g * S + qlo: g * S + qlo + 512],
                                    start=(g == 0), stop=(g == NMT - 1))
                            osb = outs.tile([128, 512], BF, tag="osb")
                            nc.vector.tensor_copy(osb[:], pso[:])
                            nc.sync.dma_start(
                                out_pt[128 * mt:128 * mt + 128,
                                       qlo:qlo + 512], osb[:])
                    return emit

                for c in range(4):            # q chunk [512c, 512c+512)
                    qlo = 512 * c
                    for g in range(NMT):      # head pair (2g, 2g+1)
                        gS = g * S
                        ktmax = 4 * c + 3
                        st8 = {}
                        for kt in range(ktmax, -1, -1):
                            q0 = max(128 * kt, qlo)
                            r0 = q0 - qlo
                            pss = psS.tile([128, 1024], F32, tag="pss")
                            # scores: two heads on PE row halves (row-tiled)
                            nc.tensor.matmul(
                                pss[:, r0:512],
                                krt[0:64, gS + 128 * kt: gS + 128 * kt + 128],
                                qrt[0:64, gS + q0: gS + qlo + 512],
                                start=True, stop=True)
                            nc.tensor.matmul(
                                pss[:, 512 + r0:1024],
                                krt[64:128, gS + 128 * kt: gS + 128 * kt + 128],
                                qrt[64:128, gS + q0: gS + qlo + 512],
                                start=True, stop=True)
                            exi = exps.tile([128, 1024], I16, tag="ex")
                            ex = exi[:].bitcast(BF)
                            src3 = pss[:].rearrange("p (h n) -> p h n",
                                                    h=2)[:, :, r0:512]
                            nc.scalar.activation(
                                ex.rearrange("p (h n) -> p h n",
                                             h=2)[:, :, r0:512],
                                src3, AF.Exp, scale=0.125)
                            if 128 * kt >= qlo:
                                # triangle on the diagonal 128-col block
                                d0 = 128 * kt - qlo
                                nc.vector.tensor_tensor(
                                    ex[:, d0:d0 + 128], ex[:, d0:d0 + 128],
                                    tri[:], ALU.mult)
                                nc.vector.tensor_tensor(
                                    ex[:, 512 + d0:512 + d0 + 128],
                                    ex[:, 512 + d0:512 + d0 + 128],
                                    tri[:], ALU.mult)
                            pend.append(make_ctx(kt, r0, ex, st8, g, ktmax))
                            if len(pend) > CTX_LAG:
                                pend.pop(0)()
                        pend.append(make_den(st8, gS, qlo))
                        if not xp:
                            while pend:
                                pend.pop(0)()
                    pend.append(make_outproj(qlo))
                    if not xp:
                        while pend:
                            pend.pop(0)()
                while pend:
                    pend.pop(0)()

    nc.finalize()
    return nc


def host_prep(query, key, value, w_q, b_q, w_k, b_k, w_v, b_v, w_o):
    """Build the 8 per-core input maps (numpy, bf16 operands)."""
    f32 = np.float32
    bf16 = mybir.dt.np(BF)
    # rope tables
    inv_freq = 1.0 / (10000.0 ** (np.arange(0, DK, 2, dtype=np.float64) / DK))
    t = np.arange(S, dtype=np.float64)
    freqs = np.outer(t, inv_freq)                       # [S, 32]
    emb = np.concatenate([freqs, freqs], axis=-1)       # [S, 64]
    cos_tab = np.cos(emb).astype(f32)                   # [S, 64]
    sin_tab = np.sin(emb).astype(f32)
    cost = np.ascontiguousarray(np.tile(cos_tab.T, (2, 1))).astype(bf16)
    sint = np.ascontiguousarray(np.tile(sin_tab.T, (2, 1))).astype(bf16)
    # rope permutation: xrot[j] = -x[2j+1] (j<32); x[2(j-32)] (j>=32)
    P = np.zeros((DK, DK), f32)
    for j in range(32):
        P[j, 2 * j + 1] = -1.0
        P[32 + j, 2 * j] = 1.0
    Pblk = np.zeros((128, 128), f32)
    Pblk[:64, :64] = P
    Pblk[64:, 64:] = P
    pmat = np.ascontiguousarray(Pblk.T).astype(bf16)
    trim = np.tril(np.ones((128, 128), f32)).T          # tri[k, r] = 1 iff k <= r
    trim = np.ascontiguousarray(trim).astype(bf16)

    in_maps = []
    for c in range(8):
        b = c // 2
        h0 = HC * (c % 2)
        sl = slice(DK * h0, DK * h0 + HD)
        wo_slice = w_o[:, sl].T.astype(f32)             # [512, 1024]
        wot = np.ascontiguousarray(
            np.concatenate([wo_slice[128 * g:128 * (g + 1), :] for g in range(NMT)],
                           axis=1)).astype(bf16)        # [128, 4*1024]
        in_maps.append({
            "xt_q": np.ascontiguousarray(query[b].T).astype(bf16),
            "xt_k": np.ascontiguousarray(key[b].T).astype(bf16),
            "xt_v": np.ascontiguousarray(value[b].T).astype(bf16),
            "wqt": np.ascontiguousarray(w_q[sl, :].T).astype(bf16),
            "wkt": np.ascontiguousarray(w_k[sl, :].T).astype(bf16),
            "wvt": np.ascontiguousarray(w_v[sl, :].T).astype(bf16),
            "bqp": np.ascontiguousarray(b_q[sl].reshape(NMT, 128).T).astype(f32),
            "bkp": np.ascontiguousarray(b_k[sl].reshape(NMT, 128).T).astype(f32),
            "bvp": np.ascontiguousarray(b_v[sl][None, :]).astype(bf16),
            "wot": wot,
            "pmat": pmat,
            "cost": cost,
            "sint": sint,
            "trim": trim,
        })
    return in_maps


def assemble(results, b_o):
    """Sum per-core transposed partials into the full [B, S, D] output."""
    out = np.zeros((B, S, D), np.float32)
    for c in range(8):
        out[c // 2] += results[c]["out_pt"].T.astype(np.float32)
    out += b_o.astype(np.float32)
    return out


_CACHE = {}


def kernel(query, key, value, mask, w_q, b_q, w_k, b_k, w_v, b_v, w_o, b_o):
    import numpy as _np
    from concourse.bass_utils import run_bass_kernel_spmd

    query = _np.asarray(query, dtype=_np.float32)
    key = _np.asarray(key, dtype=_np.float32)
    value = _np.asarray(value, dtype=_np.float32)
    w_q = _np.asarray(w_q, dtype=_np.float32)
    w_k = _np.asarray(w_k, dtype=_np.float32)
    w_v = _np.asarray(w_v, dtype=_np.float32)
    w_o = _np.asarray(w_o, dtype=_np.float32)
    b_q = _np.asarray(b_q, dtype=_np.float32)
    b_k = _np.asarray(b_k, dtype=_np.float32)
    b_v = _np.asarray(b_v, dtype=_np.float32)
    b_o = _np.asarray(b_o, dtype=_np.float32)

    if "nc" not in _CACHE:
        _CACHE["nc"] = build_nc()
    nc = _CACHE["nc"]

    in_maps = host_prep(query, key, value, w_q, b_q, w_k, b_k, w_v, b_v, w_o)
    res = run_bass_kernel_spmd(nc, in_maps, core_ids=list(range(8)))
    return assemble(res.results, b_o)


# revision 54
# speedup vs baseline: 1.1485x; 1.1485x over previous
"""Trainium2 Bass kernel for nn_MultiHeadAttention (B=4, S=2048, D=1024, H=16).

Sharding: core c in 0..7 handles batch c//2 and heads 8*(c%2) .. +8
(data-parallel over batch x tensor-parallel over heads, Megatron-style:
w_q/w_k/w_v row-split, w_o column-split; per-core partial outputs are
summed pairwise on the host, which also adds b_o).

v2: bf16 operand paths everywhere (DMA/DVE halved, no f32r small-N matmul
penalty), row-tiled score matmuls (two heads concurrently on PE row
halves), exact-causal ctx matmuls (start=True clears the whole PSUM bank,
so the partial diagonal matmul is legal), merged two-head exp via 3D APs,
denominators via reciprocal_approx_fast straight off PSUM plus a DMA
partition-broadcast.
"""

import numpy as np
import concourse.bass as bass
import concourse.tile as tile
from concourse import mybir, bacc

F32 = mybir.dt.float32
F32R = mybir.dt.float32r
BF = mybir.dt.bfloat16
I16 = mybir.dt.int16

# Schraudolph exp on DVE: bf16 bits of exp(s/8) ~= s * 23.0831 + 16250.5
# (128*log2(e)/8 and 128*(127 - 0.043), written as int16, bitcast to bf16).
EXP_A = 23.0831204
EXP_B = 16250.5
import os as _os
CTX_LAG = int(_os.environ.get("MHA_CTX_LAG", "8"))  # deferred-work queue depth
FUSE_AB = _os.environ.get("MHA_FUSE_AB", "0") == "1"  # share psS across phases
AF = mybir.ActivationFunctionType
ALU = mybir.AluOpType

B, S, D, H = 4, 2048, 1024, 16
DK = D // H          # 64
HC = 8               # heads per core
HD = HC * DK         # 512 head dims per core
KT = S // 128        # 16 k(seq) tiles
NWIN = S // 512      # 4 seq windows of 512
NMT = HD // 128      # 4 M-tiles for QKV projections (= head-pair blocks)
DMT = D // 128       # 8 M-tiles for output projection
VW = HD + HC         # 520: vaug row stride per kt (65 per head)


def build_nc(reps=1, timing=False, phases=(1, 1), probe=()):
    nc = bacc.Bacc(None, target_bir_lowering=False)

    if timing:
        # timing-only build: big IO becomes internal DRAM (no host transfer),
        # tiny dummy external in/out keep the NEFF well-formed
        def declare(name, shape, dtype, isOutput=False):
            return nc.dram_tensor(name, shape, dtype)
        dummy_in = nc.declare_dram_parameter("dummy_in", [128, 128], F32,
                                             isOutput=False)
        dummy_out = nc.declare_dram_parameter("dummy_out", [128, 128], F32,
                                              isOutput=True)
    else:
        declare = nc.declare_dram_parameter

    xt_q = declare("xt_q", [D, S], BF, isOutput=False)
    xt_k = declare("xt_k", [D, S], BF, isOutput=False)
    xt_v = declare("xt_v", [D, S], BF, isOutput=False)
    wqt = declare("wqt", [D, HD], BF, isOutput=False)
    wkt = declare("wkt", [D, HD], BF, isOutput=False)
    wvt = declare("wvt", [D, HD], BF, isOutput=False)
    bqp = declare("bqp", [128, NMT], F32, isOutput=False)
    bkp = declare("bkp", [128, NMT], F32, isOutput=False)
    bvp = declare("bvp", [1, HD], BF, isOutput=False)
    wot = declare("wot", [128, NMT * D], BF, isOutput=False)
    pmat = declare("pmat", [128, 128], BF, isOutput=False)
    cost = declare("cost", [128, S], BF, isOutput=False)
    sint = declare("sint", [128, S], BF, isOutput=False)
    trim = declare("trim", [128, 128], BF, isOutput=False)
    out_pt = declare("out_pt", [D, S], BF, isOutput=True)

    with tile.TileContext(nc) as tc:
      if timing:
          with tc.tile_pool(name="dummy", bufs=1) as dp:
              dt_ = dp.tile([128, 128], F32, tag="dt_")
              nc.sync.dma_start(dt_[:], dummy_in[:])
              nc.sync.dma_start(dummy_out[:], dt_[:])
      for _rep in range(reps):
        with tc.tile_pool(name="pers", bufs=1) as pers:
            qrt = pers.tile([128, NMT * S], BF, tag="qrt")
            krt = pers.tile([128, NMT * S], BF, tag="krt")
            vaug = pers.tile([128, KT * VW], BF, tag="vaug")    # 16*520
            ctxt = pers.tile([128, NMT * S], BF, tag="ctxt")
            tri = pers.tile([128, 128], BF, tag="tri")
            nc.sync.dma_start(tri[:], trim[:])
            pm_sb = pers.tile([128, 128], BF, tag="pm_sb")
            nc.sync.dma_start(pm_sb[:], pmat[:])
            ones_bf = pers.tile([128, 128], BF, tag="ones_bf")
            nc.vector.memset(ones_bf[:], 1.0)
            ones_f = pers.tile([1, 64], F32, tag="ones_f")
            nc.vector.memset(ones_f[:], 1.0)
            ones_r = pers.tile([1, 64], F32R, tag="ones_r")
            nc.vector.tensor_copy(ones_r[:], ones_f[:])
            wo_sb = pers.tile([128, NMT * D], BF, tag="wo_sb")
            nc.scalar.dma_start(wo_sb[:], wot[:])
            cos_sb = pers.tile([128, S], BF, tag="cos_sb")
            nc.sync.dma_start(cos_sb[:], cost[:])
            sin_sb = pers.tile([128, S], BF, tag="sin_sb")
            nc.scalar.dma_start(sin_sb[:], sint[:])

            # ---------------- phase B pools (opened early when fused) ----
            from contextlib import ExitStack
            _pb = int(_os.environ.get("MHA_POOL_BUMP", "0"))
            bst = ExitStack()

            def open_b_pools():
                d = {}
                d["exps"] = bst.enter_context(tc.tile_pool(
                    name="exps", bufs=CTX_LAG + (4 if FUSE_AB else 6)))
                d["recp"] = bst.enter_context(tc.tile_pool(name="recp", bufs=2 + _pb))
                d["sgp"] = bst.enter_context(tc.tile_pool(name="sgp", bufs=2 + _pb))
                d["outs"] = bst.enter_context(tc.tile_pool(
                    name="outs", bufs=(2 if FUSE_AB else 3) + _pb))
                d["psS"] = bst.enter_context(tc.tile_pool(name="psS", bufs=2, space="PSUM"))
                d["psC"] = bst.enter_context(tc.tile_pool(name="psC", bufs=1, space="PSUM"))
                d["psB"] = bst.enter_context(tc.tile_pool(name="psB", bufs=2, space="PSUM"))
                return d

            bp = open_b_pools() if FUSE_AB else None

            # ---------------- phase A: projections + rope ----------------
            if phases[0]:
              with ExitStack() as ast:
                wts = ast.enter_context(tc.tile_pool(name="wts", bufs=1))
                xvp = ast.enter_context(tc.tile_pool(name="xv", bufs=2))
                xqp = ast.enter_context(tc.tile_pool(name="xq", bufs=2))
                xkp = ast.enter_context(tc.tile_pool(name="xk", bufs=2))
                tmpA = ast.enter_context(tc.tile_pool(
                    name="tmpA", bufs=3 if FUSE_AB else 4))
                if FUSE_AB:
                    def ps_alloc(tag):
                        full = bp["psS"].tile([128, 1024], F32, tag="pss")
                        return full[:, 0:512]
                else:
                    psA = ast.enter_context(
                        tc.tile_pool(name="psA", bufs=2, space="PSUM"))
                    psQ = ast.enter_context(
                        tc.tile_pool(name="psQ", bufs=2, space="PSUM"))
                    psP = ast.enter_context(
                        tc.tile_pool(name="psP", bufs=2, space="PSUM"))

                    def ps_alloc(tag, _m={}):
                        pool = {"psV": psA, "psq": psQ, "psp": psP}[tag]
                        full = pool.tile([128, 512], F32, tag=tag)
                        return full[:]
                wv_sb = wts.tile([128, 8 * HD], BF, tag="wv_sb")
                wq_sb = wts.tile([128, 8 * HD], BF, tag="wq_sb")
                wk_sb = wts.tile([128, 8 * HD], BF, tag="wk_sb")
                for w_sb, wt in ((wv_sb, wvt), (wq_sb, wqt), (wk_sb, wkt)):
                    nc.scalar.dma_start(
                        w_sb[:].rearrange("p (k c) -> p k c", k=8),
                        wt[:].rearrange("(k p) c -> p k c", p=128))
                bv_sb = wts.tile([1, HD], BF, tag="bv_sb")
                nc.sync.dma_start(bv_sb[:], bvp[:])
                bq_sb = wts.tile([128, NMT], F32, tag="bq_sb")
                nc.sync.dma_start(bq_sb[:], bqp[:])
                bk_sb = wts.tile([128, NMT], F32, tag="bk_sb")
                nc.sync.dma_start(bk_sb[:], bkp[:])

                for win in range(NWIN):
                    # ---- V: natural layout [seq, hd] + ones cols ----
                    xv = xvp.tile([128, 8 * 512], BF, tag="xv")
                    dma_v = nc.sync if "one_q" in probe else (nc.scalar if win % 2 else nc.sync)
                    dma_v.dma_start(
                        xv[:].rearrange("p (k c) -> p k c", k=8),
                        xt_v[:, win * 512:(win + 1) * 512].rearrange(
                            "(k p) c -> p k c", p=128))
                    for st in range(4):
                        ps = ps_alloc("psV")
                        for k in range(8):
                            nc.tensor.matmul(
                                ps[:],
                                xv[:, k * 512 + st * 128: k * 512 + st * 128 + 128],
                                wv_sb[:, k * HD:(k + 1) * HD],
                                start=(k == 0), stop=False)
                        # + b_v broadcast along seq via ones outer product
                        nc.tensor.matmul(ps[:], ones_bf[0:1, 0:128], bv_sb[:],
                                         start=False, stop=True)
                        base = (win * 4 + st) * VW
                        nc.vector.tensor_copy(
                            vaug[:, base: base + VW].rearrange(
                                "p (h c) -> p h c", c=65)[:, :, 0:64],
                            ps[:].rearrange("p (h c) -> p h c", c=64))
                        nc.vector.tensor_copy(
                            vaug[:, base + 64: base + VW: 65],
                            ones_bf[:, 0:HC])

                    # ---- Q and K: transposed layout + rope ----
                    for qk, (xt, xp, w_sb, b_sb, dst) in enumerate((
                            (xt_q, xqp, wq_sb, bq_sb, qrt),
                            (xt_k, xkp, wk_sb, bk_sb, krt))):
                        xq = xp.tile([128, 8 * 512], BF, tag="xq")
                        dma_x = nc.sync if "one_q" in probe else (nc.sync if (win + qk) % 2 else nc.scalar)
                        dma_x.dma_start(
                            xq[:].rearrange("p (k c) -> p k c", k=8),
                            xt[:, win * 512:(win + 1) * 512].rearrange(
                                "(k p) c -> p k c", p=128))
                        for mt in range(NMT):
                            psq = ps_alloc("psq")
                            for k in range(8):
                                nc.tensor.matmul(
                                    psq[:],
                                    w_sb[:, k * HD + 128 * mt: k * HD + 128 * mt + 128],
                                    xq[:, k * 512:(k + 1) * 512],
                                    start=(k == 0), stop=(k == 7))
                            qt_sb = tmpA.tile([128, 512], BF, tag="qt_sb")
                            nc.scalar.activation(qt_sb[:], psq[:], AF.Identity,
                                                 bias=b_sb[:, mt:mt + 1],
                                                 scale=1.0)
                            psp = ps_alloc("psp")
                            nc.tensor.matmul(psp[:], pm_sb[:], qt_sb[:],
                                             start=True, stop=True)
                            t1 = tmpA.tile([128, 512], BF, tag="t1")
                            nc.vector.tensor_tensor(
                                t1[:], qt_sb[:],
                                cos_sb[:, win * 512:(win + 1) * 512], ALU.mult)
                            t2 = tmpA.tile([128, 512], BF, tag="t2")
                            nc.vector.tensor_tensor(
                                t2[:], psp[:],
                                sin_sb[:, win * 512:(win + 1) * 512], ALU.mult)
                            nc.vector.tensor_tensor(
                                dst[:, mt * S + win * 512: mt * S + (win + 1) * 512],
                                t1[:], t2[:], ALU.add)

            # ---------------- phase B: attention + output projection ----------------
            if phases[1]:
              if bp is None:
                  bp = open_b_pools()
              exps, recp, sgp, outs = bp["exps"], bp["recp"], bp["sgp"], bp["outs"]
              psS, psC, psB = bp["psS"], bp["psC"], bp["psB"]
              if True:
                # Deferred-work queue: ctx matmuls, den/normalize chains and
                # out-projections are emitted CTX_LAG score-slots late; with
                # "xp" the queue carries across pairs/chunks so PE fills its
                # Act-gated score-stream gaps with the previous pair's work.
                xp = "noxp" not in probe
                pend = []

                def make_ctx(kt, r0, ex, st8, g, ktmax):
                    def emit():
                        if "psc" not in st8:
                            psc_t = psC.tile([65, 1024], F32, tag="psc")
                            st8["psc"] = psc_t
                        psc = st8["psc"]
                        nc.tensor.matmul(
                            psc[0:65, r0:512],
                            vaug[:, kt * VW + 65 * 2 * g:
                                 kt * VW + 65 * 2 * g + 65],
                            ex[:, r0:512],
                            start=(kt == ktmax), stop=(kt == 0))
                        nc.tensor.matmul(
                            psc[0:65, 512 + r0:1024],
                            vaug[:, kt * VW + 65 * (2 * g + 1):
                                 kt * VW + 65 * (2 * g + 1) + 65],
                            ex[:, 512 + r0:1024],
                            start=(kt == ktmax), stop=(kt == 0))
                    return emit

                def make_den(st8, gS, qlo):
                    def emit():
                        psc = st8["psc"]
                        den_r = recp.tile([1, 1024], F32R, tag="den_r")
                        nc.vector.tensor_copy(den_r[:], psc[64:65, :])
                        bcA = psB.tile([64, 512], F32, tag="bc")
                        nc.tensor.matmul(bcA[:], ones_r[:],
                                         den_r[0:1, 0:512],
                                         start=True, stop=True)
                        bcB = psB.tile([64, 512], F32, tag="bc")
                        nc.tensor.matmul(bcB[:], ones_r[:],
                                         den_r[0:1, 512:1024],
                                         start=True, stop=True)
                        rbA = sgp.tile([64, 512], F32, tag="rbA")
                        nc.vector.reciprocal_approx_fast(rbA[:], bcA[:])
                        rbB = sgp.tile([64, 512], F32, tag="rbB")
                        nc.vector.reciprocal_approx_fast(rbB[:], bcB[:])
                        nc.vector.tensor_tensor(
                            ctxt[0:64, gS + qlo: gS + qlo + 512],
                            psc[0:64, 0:512], rbA[:], ALU.mult)
                        sg = sgp.tile([64, 512], BF, tag="sg")
                        nc.vector.tensor_tensor(
                            sg[:], psc[0:64, 512:1024], rbB[:], ALU.mult)
                        nc.sync.dma_start(
                            ctxt[64:128, gS + qlo: gS + qlo + 512], sg[:])
                    return emit

                def make_outproj(qlo):
                    def emit():
                        for mt in range(DMT):
                            pso_full = psS.tile([128, 1024], F32, tag="pss")
                            pso = pso_full[:, 0:512]
                            for g in range(NMT):
                                nc.tensor.matmul(
                                    pso[:],
                                    wo_sb[:, g * D + 128 * mt:
                                          g * D + 128 * mt + 128],
                                    ctxt[:, g * S + qlo: g * S + qlo + 512],
                                    start=(g == 0), stop=(g == NMT - 1))
                            osb = outs.tile([128, 512], BF, tag="osb")
                            nc.vector.tensor_copy(osb[:], pso[:])
                            nc.sync.dma_start(
                                out_pt[128 * mt:128 * mt + 128,
                                       qlo:qlo + 512], osb[:])
                    return emit

                for c in range(4):            # q chunk [512c, 512c+512)
                    qlo = 512 * c
                    for g in range(NMT):      # head pair (2g, 2g+1)
                        gS = g * S
                        ktmax = 4 * c + 3
                        st8 = {}
                        for kt in range(ktmax, -1, -1):
                            q0 = max(128 * kt, qlo)
                            r0 = q0 - qlo
                            pss = psS.tile([128, 1024], F32, tag="pss")
                            # scores: two heads on PE row halves (row-tiled)
                            nc.tensor.matmul(
                                pss[:, r0:512],
                                krt[0:64, gS + 128 * kt: gS + 128 * kt + 128],
                                qrt[0:64, gS + q0: gS + qlo + 512],
                                start=True, stop=True)
                            nc.tensor.matmul(
                                pss[:, 512 + r0:1024],
                                krt[64:128, gS + 128 * kt: gS + 128 * kt + 128],
                                qrt[64:128, gS + q0: gS + qlo + 512],
                                start=True, stop=True)
                            exi = exps.tile([128, 1024], I16, tag="ex")
                            ex = exi[:].bitcast(BF)
                            src3 = pss[:].rearrange("p (h n) -> p h n",
                                                    h=2)[:, :, r0:512]
                            nc.scalar.activation(
                                ex.rearrange("p (h n) -> p h n",
                                             h=2)[:, :, r0:512],
                                src3, AF.Exp, scale=0.125)
                            if 128 * kt >= qlo:
                                # triangle on the diagonal 128-col block
                                d0 = 128 * kt - qlo
                                nc.vector.tensor_tensor(
                                    ex[:, d0:d0 + 128], ex[:, d0:d0 + 128],
                                    tri[:], ALU.mult)
                                nc.vector.tensor_tensor(
                                    ex[:, 512 + d0:512 + d0 + 128],
                                    ex[:, 512 + d0:512 + d0 + 128],
                                    tri[:], ALU.mult)
                            pend.append(make_ctx(kt, r0, ex, st8, g, ktmax))
                            if len(pend) > CTX_LAG:
                                pend.pop(0)()
                        pend.append(make_den(st8, gS, qlo))
                        if not xp:
                            while pend:
                                pend.pop(0)()
                    pend.append(make_outproj(qlo))
                    if not xp:
                        while pend:
                            pend.pop(0)()
                while pend:
                    pend.pop(0)()
            bst.close()

    nc.finalize()
    return nc


def host_prep(query, key, value, w_q, b_q, w_k, b_k, w_v, b_v, w_o):
    """Build the 8 per-core input maps (numpy, bf16 operands)."""
    f32 = np.float32
    bf16 = mybir.dt.np(BF)
    # rope tables
    inv_freq = 1.0 / (10000.0 ** (np.arange(0, DK, 2, dtype=np.float64) / DK))
    t = np.arange(S, dtype=np.float64)
    freqs = np.outer(t, inv_freq)                       # [S, 32]
    emb = np.concatenate([freqs, freqs], axis=-1)       # [S, 64]
    cos_tab = np.cos(emb).astype(f32)                   # [S, 64]
    sin_tab = np.sin(emb).astype(f32)
    cost = np.ascontiguousarray(np.tile(cos_tab.T, (2, 1))).astype(bf16)
    sint = np.ascontiguousarray(np.tile(sin_tab.T, (2, 1))).astype(bf16)
    # rope permutation: xrot[j] = -x[2j+1] (j<32); x[2(j-32)] (j>=32)
    P = np.zeros((DK, DK), f32)
    for j in range(32):
        P[j, 2 * j + 1] = -1.0
        P[32 + j, 2 * j] = 1.0
    Pblk = np.zeros((128, 128), f32)
    Pblk[:64, :64] = P
    Pblk[64:, 64:] = P
    pmat = np.ascontiguousarray(Pblk.T).astype(bf16)
    trim = np.tril(np.ones((128, 128), f32)).T          # tri[k, r] = 1 iff k <= r
    trim = np.ascontiguousarray(trim).astype(bf16)

    in_maps = []
    for c in range(8):
        b = c // 2
        h0 = HC * (c % 2)
        sl = slice(DK * h0, DK * h0 + HD)
        wo_slice = w_o[:, sl].T.astype(f32)             # [512, 1024]
        wot = np.ascontiguousarray(
            np.concatenate([wo_slice[128 * g:128 * (g + 1), :] for g in range(NMT)],
                           axis=1)).astype(bf16)        # [128, 4*1024]
        in_maps.append({
            "xt_q": np.ascontiguousarray(query[b].T).astype(bf16),
            "xt_k": np.ascontiguousarray(key[b].T).astype(bf16),
            "xt_v": np.ascontiguousarray(value[b].T).astype(bf16),
            "wqt": np.ascontiguousarray(w_q[sl, :].T).astype(bf16),
            "wkt": np.ascontiguousarray(w_k[sl, :].T).astype(bf16),
            "wvt": np.ascontiguousarray(w_v[sl, :].T).astype(bf16),
            "bqp": np.ascontiguousarray(b_q[sl].reshape(NMT, 128).T).astype(f32),
            "bkp": np.ascontiguousarray(b_k[sl].reshape(NMT, 128).T).astype(f32),
            "bvp": np.ascontiguousarray(b_v[sl][None, :]).astype(bf16),
            "wot": wot,
            "pmat": pmat,
            "cost": cost,
            "sint": sint,
            "trim": trim,
        })
    return in_maps


def assemble(results, b_o):
    """Sum per-core transposed partials into the full [B, S, D] output."""
    out = np.zeros((B, S, D), np.float32)
    for c in range(8):
        out[c // 2] += results[c]["out_pt"].T.astype(np.float32)
    out += b_o.astype(np.float32)
    return out


_CACHE = {}


def kernel(query, key, value, mask, w_q, b_q, w_k, b_k, w_v, b_v, w_o, b_o):
    import numpy as _np
    from concourse.bass_utils import run_bass_kernel_spmd

    query = _np.asarray(query, dtype=_np.float32)
    key = _np.asarray(key, dtype=_np.float32)
    value = _np.asarray(value, dtype=_np.float32)
    w_q = _np.asarray(w_q, dtype=_np.float32)
    w_k = _np.asarray(w_k, dtype=_np.float32)
    w_v = _np.asarray(w_v, dtype=_np.float32)
    w_o = _np.asarray(w_o, dtype=_np.float32)
    b_q = _np.asarray(b_q, dtype=_np.float32)
    b_k = _np.asarray(b_k, dtype=_np.float32)
    b_v = _np.asarray(b_v, dtype=_np.float32)
    b_o = _np.asarray(b_o, dtype=_np.float32)

    if "nc" not in _CACHE:
        _CACHE["nc"] = build_nc()
    nc = _CACHE["nc"]

    in_maps = host_prep(query, key, value, w_q, b_q, w_k, b_k, w_v, b_v, w_o)
    res = run_bass_kernel_spmd(nc, in_maps, core_ids=list(range(8)))
    return assemble(res.results, b_o)
